# revision 1
# baseline (speedup 1.0000x reference)
"""Trainium2 Bass kernel for nn_AttnCalc (coverage attention).

Contract: kernel(**inputs) takes FULL unsharded numpy inputs, distributes
batch-parallel across 8 NeuronCores, returns the full
(context_vector, attn_weights, new_coverage) tuple like the reference.

Math per batch b:
  enc_feat = enc[b] @ attn_w.T + attn_b          [L,H]
  dec_feat = dec_w @ hidden[b] + dec_b           [H]
  cov_feat = w_eff @ coverage[b] + cvg_b         [L]   (w_eff = cvg_w[:,:,0,(H-1)//2])
  feats    = tanh(enc_feat + dec_feat + cov_feat[:,None])
  scores   = feats @ v[b]  (masked, softmax over L) -> aw
  new_cov  = coverage[b] + aw
  context  = aw @ enc[b]                         [H]

fp16 datapath with fp32 PSUM accumulation; all heavy contractions on the
PE array (1 cycle/row fp16):
  enc_feat:  eT tiles [128 (H-chunk), L]; contraction over H.  cov_feat
             is a K=1 rank-1 matmul folded into the PSUM accumulation;
             dec_feat(+biases, host-precomputed) ride the tanh bias and
             tanh reads PSUM directly.
  scores:    v-column lhsT against fp16 feats.
  context:   aw transposed to columns (aw4[p,k]=aw[4p+k], via a DMA
             round-trip through the aw output row) against a second enc
             layout encN[k][p,h]=enc[4p+k,h]; ctx row accumulates in
             PSUM fp32 and is copied out by the Scalar engine.

Three-stage software pipeline per iteration it:
  produce(b=it):    prefetch eT/eN(b+1), enc+cov matmuls(b), tanh(b)
  softmax(b=it-1):  scores matmul, masked softmax, aw out + aw4 column
                    load + ncov out on the Pool DMA queue
  context(b=it-2):  4 PE matmuls + Scalar copy of the ctx row
eT loads own the SP queue, encN loads the DVE queue, outputs the Pool
queue, so output DMAs never delay prefetch (keeps the PE p-state
ramped at 2.4 GHz).

The target walrus build allows only ONE semaphore wait per TPB compute
instruction, so tiny "absorber" ops (1x1 matmul / copy) pick up extra
waits ahead of real work, and _legalize_waits redistributes the rest.
"""

import sys
import os

sys.path.insert(0, "/opt/trn_rl_repo")

import numpy as np

import concourse.bass as bass
import concourse.tile as tile
from concourse import mybir
from concourse.bass_utils import run_bass_kernel_spmd
from concourse.tile_rust import add_dep_helper

B, L, H = 64, 512, 512
NCORES = 8
BLOC = B // NCORES          # batches per core
P = 128                     # SBUF partitions
PC = H // P                 # 128-chunks along H (== along L)
F32 = mybir.dt.float32
F16 = mybir.dt.float16
Tanh = mybir.ActivationFunctionType.Tanh
Exp = mybir.ActivationFunctionType.Exp
Copy = mybir.ActivationFunctionType.Copy

_CACHE = {}


def _build_program():
    nc = bass.Bass()

    encT = nc.declare_dram_parameter("encT", [BLOC, P, PC, L], F16,
                                     isOutput=False)
    encN = nc.declare_dram_parameter("encN", [BLOC, P, PC, H], F16,
                                     isOutput=False)
    attn_wPK = nc.declare_dram_parameter("attn_wPK", [P, PC, H], F16,
                                         isOutput=False)
    vT = nc.declare_dram_parameter("vT", [H, BLOC], F16, isOutput=False)
    cov_in = nc.declare_dram_parameter("cov_in", [BLOC, L], F32, isOutput=False)
    maskb = nc.declare_dram_parameter("maskb", [BLOC, L], F32, isOutput=False)
    # host-precomputed small linears (0.2% of the FLOPs):
    #   covf16[b, l]  = (w_eff @ coverage[b] + cvg_b)[l]           fp16 rows
    #   biasPE[p,o,b] = (dec_w @ hidden[b] + dec_b + attn_b)[o*128+p]
    covf16 = nc.declare_dram_parameter("covf16", [BLOC, L], F16, isOutput=False)
    biasPE = nc.declare_dram_parameter("biasPE", [P, PC, BLOC], F32,
                                       isOutput=False)

    aw_out = nc.declare_dram_parameter("aw_out", [BLOC, L], F16, isOutput=True)
    ncov_out = nc.declare_dram_parameter("ncov_out", [BLOC, L], F32, isOutput=True)
    ctx_out = nc.declare_dram_parameter("ctx_out", [BLOC, H], F32, isOutput=True)

    def row3(dram2d, b=BLOC):
        # [BLOC, L] dram -> [1, BLOC, L] AP so rows can live on partition 0
        return dram2d[:, :].rearrange("b l -> (b l)")[None].rearrange(
            "o (b l) -> o b l", b=b)

    with tile.TileContext(nc) as tc:
        with (
            tc.tile_pool(name="const", bufs=1) as const,
            tc.tile_pool(name="enc", bufs=4) as epool,
            tc.tile_pool(name="encn", bufs=5) as npool,
            tc.tile_pool(name="feat", bufs=3) as fpool,
            tc.tile_pool(name="aw4", bufs=4) as apool,
            tc.tile_pool(name="eps", bufs=3, space=bass.MemorySpace.PSUM) as ppool,
            tc.tile_pool(name="scps", bufs=2, space=bass.MemorySpace.PSUM) as scpool,
            tc.tile_pool(name="cxps", bufs=2, space=bass.MemorySpace.PSUM) as cxpool,
            tc.tile_pool(name="dumps", bufs=1, space=bass.MemorySpace.PSUM) as dumpool,
        ):
            # -------- wait absorbers --------
            dum_t = dumpool.tile([1, 64], F32, tag="dummy")
            dve_dum = const.tile([1, 256], F32)
            act_dum = const.tile([1, 256], F32)
            _ctr = {"pe": 0, "dve": 0, "act": 0}

            def pe_abs(ap):
                i = _ctr["pe"] = (_ctr["pe"] + 1) % 64
                if ap.dtype not in (F32, F16):
                    ap = ap.bitcast(F32)
                return nc.tensor.matmul(dum_t[0:1, i:i + 1], ap, ap,
                                        start=True, stop=True)

            def dve_abs(ap):
                i = _ctr["dve"] = (_ctr["dve"] + 1) % 256
                return nc.vector.tensor_copy(dve_dum[0:1, i:i + 1], ap)

            def act_abs(ap):
                i = _ctr["act"] = (_ctr["act"] + 1) % 256
                return nc.scalar.activation(act_dum[0:1, i:i + 1], ap, Copy)

            def pin(real, *deps):
                for d in deps:
                    add_dep_helper(real.ins, d.ins, sync=False,
                                   reason="absorber ordering")

            # ---------------- constants ----------------
            # SP queue carries only the PE-critical stream: wA then eT loads.
            wA = const.tile([P, PC, H], F16)   # attn_wT  [h=k*128+p][o]
            vS = const.tile([P, PC, BLOC], F16)
            wA_dma = nc.sync.dma_start(out=wA, in_=attn_wPK[:, :, :])
            # Pool queue: small consume-side constants, in first-use order.
            cov16r = const.tile([1, BLOC, L], F16)  # cov_feat rows (fp16)
            bias_sb = const.tile([P, PC, BLOC], F32)
            nc.gpsimd.dma_start(out=cov16r, in_=row3(covf16))
            nc.gpsimd.dma_start(out=bias_sb, in_=biasPE[:, :, :])
            nc.gpsimd.dma_start(out=vS, in_=vT[:, :].rearrange("(k p) b -> p k b", p=P))
            mb = const.tile([1, BLOC, L], F32)
            covin = const.tile([1, BLOC, L], F32)
            nc.gpsimd.dma_start(out=mb, in_=row3(maskb))
            nc.gpsimd.dma_start(out=covin, in_=row3(cov_in))

            ones_b = const.tile([1, BLOC], F32)
            nc.vector.memset(ones_b, 1.0)
            ones16_p = const.tile([1, P], F16)
            nc.vector.memset(ones16_p, 1.0)
            d_mb = dve_abs(mb[0:1, 0, 0:1])
            d_cvn = dve_abs(covin[0:1, 0, 0:1])

            sc = const.tile([1, BLOC, L], F32)      # scores -> exp (fp32 rows)
            aw16 = const.tile([1, BLOC, L], F16)    # final aw rows (fp16)
            nmx = const.tile([1, BLOC, 1], F32)
            se = const.tile([1, BLOC, 1], F32)
            rse = const.tile([1, BLOC, 1], F32)
            ctxr = const.tile([1, BLOC, H], F32)    # ctx rows staging

            a_bias = act_abs(bias_sb[0:1, 0, 0:1])
            d_wA = pe_abs(wA[0:1, 0, 0:1])

            # eT/eN(0) prefetch ahead of the loop
            eT_tiles = {}
            eN_tiles = {}
            eT0 = epool.tile([P, PC, L], F16)
            eT0_dma = nc.sync.dma_start(out=eT0, in_=encT[0])
            # bulk loads form a true (semaphore) chain so the load the PE
            # needs NEXT always gets full DMA bandwidth instead of sharing
            # it with prefetches
            add_dep_helper(eT0_dma.ins, wA_dma.ins, sync=True,
                           reason="bulk chain")
            bulk = {"prev": eT0_dma}
            eT_tiles[0] = (eT0, eT0_dma)

            # ---------------- main pipeline ----------------
            px = {"exp": None}
            prev_eT = eT0_dma
            prev_eN = None
            state1 = {}   # b -> (ft,) after produce
            state2 = {}   # b -> (aw4_tile,) after softmax

            def emit_softmax(sb):
                (sft,) = state1.pop(sb)
                d_f = pe_abs(sft[0:1, 0, 0:1])
                d_vs0 = pe_abs(vS[0:1, 0, 0:1]) if sb == 0 else None
                sc_ps = scpool.tile([1, L], F32, tag="sc")
                smm = None
                for k in range(PC):
                    smm = nc.tensor.matmul(sc_ps, vS[:, k, sb:sb + 1],
                                           sft[:, k, :],
                                           start=(k == 0), stop=(k == 3))
                    if k == 0:
                        pin(smm, d_f)
                        if sb == 0:
                            pin(smm, d_vs0)

                scr = sc[0:1, sb, :]
                aw_r = aw16[0:1, sb, :]
                madd = nc.vector.tensor_add(scr, sc_ps, mb[0:1, sb, :])
                if sb == 0:
                    pin(madd, d_mb)
                nc.vector.tensor_reduce(out=nmx[0:1, sb, :], in_=scr,
                                        axis=mybir.AxisListType.X,
                                        op=mybir.AluOpType.max, negate=True)
                px["exp"] = nc.scalar.activation(
                    out=scr, in_=scr, func=Exp,
                    bias=nmx[0:1, sb, :], scale=1.0,
                    accum_out=se[0:1, sb, :])
                nc.vector.reciprocal(rse[0:1, sb, :], se[0:1, sb, :])
                awmul = nc.vector.tensor_scalar_mul(aw_r, scr,
                                                    rse[0:1, sb, :])

                # aw row out (fp16), then load back as columns
                # aw4[p, k] = aw[4p + k] for the PE context contraction
                gp_slots = [nc.gpsimd.nop(nofuse=True) for _ in range(3)]
                pin(gp_slots[0], smm)
                pin(gp_slots[1], awmul)
                pin(gp_slots[2], gp_slots[1])
                aw_dma = nc.gpsimd.dma_start(out=aw_out[sb:sb + 1, :],
                                             in_=aw_r)
                pin(aw_dma, gp_slots[2])
                aw4 = apool.tile([P, PC], F16, tag="aw4")
                a4_dma = nc.gpsimd.dma_start(
                    out=aw4,
                    in_=aw_out[sb:sb + 1, :].rearrange("o (p k) -> (o p) k",
                                                       p=P))
                pin(a4_dma, aw_dma)

                # new_coverage row (in place over covin row)
                ncadd = nc.vector.tensor_add(covin[0:1, sb, :],
                                             covin[0:1, sb, :], aw_r)
                if sb == 0:
                    pin(ncadd, d_cvn)
                gp_nc = nc.gpsimd.nop(nofuse=True)
                pin(gp_nc, ncadd)
                nc_dma = nc.gpsimd.dma_start(out=ncov_out[sb:sb + 1, :],
                                             in_=covin[0:1, sb, :])
                pin(nc_dma, gp_nc)
                state2[sb] = (aw4,)
            for it in range(BLOC + 3):
                # ---- prefetch eT/eN(it+1) ----
                if it + 1 < BLOC:
                    bn = it + 1
                    sps = [nc.sync.nop(nofuse=True) for _ in range(4)]
                    pin(sps[0], prev_eT)
                    for _j in range(1, 4):
                        pin(sps[_j], sps[_j - 1])
                    eTn = epool.tile([P, PC, L], F16)
                    eTn_dma = nc.sync.dma_start(out=eTn, in_=encT[bn])
                    pin(eTn_dma, sps[3])
                    add_dep_helper(eTn_dma.ins, bulk["prev"].ins, sync=True,
                                   reason="bulk chain")
                    bulk["prev"] = eTn_dma
                    prev_eT = eTn_dma
                    eT_tiles[bn] = (eTn, eTn_dma)
                if 1 <= it <= BLOC:
                    bn = it - 1
                    vps = [nc.scalar.nop(nofuse=True) for _ in range(2)]
                    if prev_eN is not None:
                        pin(vps[0], prev_eN)
                    pin(vps[1], vps[0])
                    eNn = npool.tile([P, PC, H], F16)
                    eNn_dma = nc.scalar.dma_start(out=eNn, in_=encN[bn])
                    pin(eNn_dma, vps[1])
                    add_dep_helper(eNn_dma.ins, bulk["prev"].ins, sync=True,
                                   reason="bulk chain")
                    bulk["prev"] = eNn_dma
                    prev_eN = eNn_dma
                    eN_tiles[bn] = eNn

                # ---- produce(b=it): enc matmuls + tanh.  The previous
                # batch's scores matmuls + softmax are emitted right after
                # the first o-chunk, so its aw round-trip overlaps the rest
                # of this batch's enc work (shorter pipeline tail). ----
                if it < BLOC:
                    b = it
                    eT, _dma = eT_tiles.pop(b)
                    d_e = pe_abs(eT[0:1, 0, 0:1])

                    a_slot = act_abs(ones_b[0:1, 0:1])
                    a_slot2 = act_abs(ones_b[0:1, 0:1])
                    if px["exp"] is not None:
                        pin(a_slot, px["exp"])
                    pin(a_slot2, a_slot)
                    ft = fpool.tile([P, PC, L], F16)
                    first_th = None
                    for o in range(PC):
                        ps = ppool.tile([P, L], F32, tag="encps")
                        for k in range(PC):
                            mm = nc.tensor.matmul(ps, wA[:, k, o * P:(o + 1) * P],
                                                  eT[:, k, :], start=(k == 0),
                                                  stop=False)
                            if k == 0:
                                pin(mm, d_e)
                                if b == 0 and o == 0:
                                    pin(mm, d_wA)
                        # cov_feat rank-1 fold: ps[:, l] += cov_feat[b][l]
                        if b == 0 and o == 0:
                            d_cov = pe_abs(cov16r[0:1, 0, 0:1])
                            d_o16p = pe_abs(ones16_p[0:1, 0:1])
                        mmc = nc.tensor.matmul(ps, ones16_p[:, :],
                                               cov16r[0:1, b, :],
                                               start=False, stop=True)
                        if b == 0 and o == 0:
                            pin(mmc, d_cov, d_o16p)
                        th = nc.scalar.activation(
                            out=ft[:, o, :], in_=ps, func=Tanh,
                            bias=bias_sb[:, o, b:b + 1], scale=1.0)
                        if first_th is None:
                            first_th = th
                            pin(th, a_slot2)
                        if b == 0 and o == 0:
                            pin(th, a_bias)
                        if o == 0 and it >= 1:
                            emit_softmax(it - 1)
                    state1[b] = (ft,)
                elif it == BLOC:
                    emit_softmax(it - 1)

                # ---- context(b=it-3): 4 PE matmuls + Scalar row copy ----
                if it >= 3:
                    b = it - 3
                    (aw4,) = state2.pop(b)
                    eN = eN_tiles.pop(b)
                    d_a4 = pe_abs(aw4[0:1, 0:1])
                    d_n = pe_abs(eN[0:1, 0, 0:1])
                    cx_ps = cxpool.tile([1, H], F32, tag="cx")
                    for k in range(PC):
                        cmm = nc.tensor.matmul(cx_ps, aw4[:, k:k + 1],
                                               eN[:, k, :],
                                               start=(k == 0), stop=(k == 3))
                        if k == 0:
                            pin(cmm, d_a4, d_n)
                    ccp = nc.scalar.copy(ctxr[0:1, b, :], cx_ps)
                    sp_cx = [nc.sync.nop(nofuse=True) for _ in range(2)]
                    pin(sp_cx[0], ccp)
                    pin(sp_cx[1], sp_cx[0])
                    cx_dma = nc.sync.dma_start(out=ctx_out[b:b + 1, :],
                                               in_=ctxr[0:1, b, :])
                    pin(cx_dma, sp_cx[1])

            sp_ct = [nc.sync.nop(nofuse=True) for _ in range(2)]
            pin(sp_ct[0], cx_dma)
            pin(sp_ct[1], sp_ct[0])

            # tail landing slots for the kernel-tail drain waits
            tail = sp_ct[1]
            for _ in range(22):
                n = nc.sync.nop(nofuse=True)
                pin(n, tail)
                tail = n

    _legalize_waits(nc)
    return nc


# The nix walrus build (setupSyncWait) accepts only ONE sync wait per TPB
# instruction (compute and DMA alike).  Tile can emit several.  Because the
# committed instruction order is a topological order of the dependency
# graph, a wait whose producing semaphore update completes at block index p
# can be safely carried by ANY same-engine instruction at index > p that
# precedes the original carrier: engines execute in order, so the original
# instruction still starts after the wait is satisfied, and the producer
# (committed before the new carrier) cannot depend on it -- no deadlock.
# Assign waits to instructions as an interval matching problem.
def _legalize_waits(nc):
    import concourse.mybir as _mb

    fn = nc.m.functions[0]
    stuck = []
    NO_LANDING = ("InstISA", "InstEventSemaphore", "InstUnconditionalBranch",
                  "InstCall", "InstRegisterMove", "InstHalt")
    insts = []
    for blk in fn.blocks:
        insts.extend(blk.instructions)

    sem_hist = {}
    cum = {}
    streams = {}
    for i, inst in enumerate(insts):
        si = inst.sync_info
        if si is not None:
            for u in si.on_update:
                cum[u.id] = cum.get(u.id, 0) + u.update_value
                sem_hist.setdefault(u.id, []).append((i, cum[u.id]))
        streams.setdefault(inst.engine, []).append(i)

    def producer_idx(w):
        hist = sem_hist.get(w.id)
        if hist is None:
            return None            # unknown semaphore: not movable
        for i, v in hist:
            if v >= w.wait_value:
                return i
        return None

    for eng, stream in streams.items():
        movable_spos = []
        pinned = {}                # spos -> unmovable waits
        waits = []                 # (carrier_spos, producer_bidx, wait)
        has_multi = False
        pos_of = {i: spos for spos, i in enumerate(stream)}
        eng_name = str(eng).split(".")[-1]
        for spos, i in enumerate(stream):
            inst = insts[i]
            si = inst.sync_info
            ws = list(si.on_wait) if si is not None else []
            if len(ws) > 1:
                has_multi = True
            # Waits on this engine's own execution-counter semaphore whose
            # producing (non-DMA) instruction ran >=8 instructions earlier
            # on this engine are redundant: engine-counter updates fire in
            # engine order, and 8 instructions is far beyond the pipeline
            # write-drain window.  DMA-completion sems fire asynchronously
            # and are never dropped.
            def _redundant(w):
                if w.ant_name.split("_")[0] != eng_name:
                    return False
                p = producer_idx(w)
                return (p is not None and p in pos_of
                        and insts[p].__class__.__name__ != "InstDMACopy"
                        and spos - pos_of[p] >= 8)
            nws = [w for w in ws if not _redundant(w)]
            if len(nws) != len(ws):
                has_multi = True
            ws = nws

            def mov(w):
                if w.wait_reg is not None or w.wait_value <= 0:
                    return False
                p = producer_idx(w)
                return p is not None and p < i
            special = inst.__class__.__name__ in NO_LANDING
            unmov = [w for w in ws if special or not mov(w)]
            if unmov:
                pinned[spos] = unmov
            elif not special:
                movable_spos.append(spos)
            if special:
                continue
            best = {}
            for w in ws:
                if not mov(w):
                    continue
                if w.id not in best or w.wait_value > best[w.id].wait_value:
                    best[w.id] = w
            for w in best.values():
                waits.append((spos, producer_idx(w), w))
        if not has_multi:
            continue
        bidx_of = {spos: stream[spos] for spos in range(len(stream))}
        free = sorted(movable_spos)
        assign = {}
        for carrier, pbidx, w in sorted(waits, key=lambda t: (t[0], -t[1])):
            chosen = None
            for spos in reversed(free):
                if spos > carrier:
                    continue
                if bidx_of[spos] <= pbidx:
                    break
                chosen = spos
                break
            if chosen is None:
                stuck.append((insts[stream[carrier]].name,
                              insts[stream[carrier]].__class__.__name__,
                              w.ant_name, w.wait_value))
                continue
            free.remove(chosen)
            assign.setdefault(chosen, []).append(w)
        for spos in range(len(stream)):
            inst = insts[stream[spos]]
            si = inst.sync_info
            ups = list(si.on_update) if si is not None else []
            new_w = pinned.get(spos, []) + assign.get(spos, [])
            if si is None and not new_w:
                continue
            inst.sync_info = _mb.SyncInfo(on_wait=new_w, on_update=ups)
    if stuck:
        raise RuntimeError(f"wait legalization failed: {stuck[:8]}")


def _get_program():
    if "nc" not in _CACHE:
        _CACHE["nc"] = _build_program()
    return _CACHE["nc"]


def _prep_core_inputs(c, enc, maskf, coverage, attn_w, v, covf, biasf):
    s = slice(c * BLOC, (c + 1) * BLOC)
    enc_l = enc[s]                                   # [BLOC, L, H]
    enc16 = enc_l.astype(np.float16)
    return {
        # encT[b, p, k, l] = enc[b, l, 128k+p]
        "encT": np.ascontiguousarray(
            enc16.transpose(0, 2, 1).reshape(BLOC, PC, P, L).transpose(0, 2, 1, 3)),
        # encN[b, p, k, h] = enc[b, 4p+k, h]  (l = 4p + k)
        "encN": np.ascontiguousarray(enc16.reshape(BLOC, P, PC, H)),
        # attn_wPK[p, k, o] = attn_w.T[128k+p, o]
        "attn_wPK": np.ascontiguousarray(
            attn_w.T.astype(np.float16).reshape(PC, P, H).transpose(1, 0, 2)),
        "vT": np.ascontiguousarray(v[s].T).astype(np.float16),
        "cov_in": np.ascontiguousarray(coverage[s]),
        "maskb": np.ascontiguousarray(maskf[s]),
        "covf16": np.ascontiguousarray(covf[s]).astype(np.float16),
        # biasPE[p, o, b] = biasf[b, o*128+p]
        "biasPE": np.ascontiguousarray(
            biasf[s].T.reshape(PC, P, BLOC).transpose(1, 0, 2)),
    }


def kernel(encoder_outputs, attn_mask, hidden, coverage,
           attn_w, attn_b, dec_w, dec_b, cvg_w, cvg_b, v):
    enc = np.asarray(encoder_outputs, dtype=np.float32)
    mask = np.asarray(attn_mask)
    hidden = np.asarray(hidden, dtype=np.float32)
    coverage = np.asarray(coverage, dtype=np.float32)
    attn_w = np.asarray(attn_w, dtype=np.float32)
    attn_b = np.asarray(attn_b, dtype=np.float32)
    dec_w = np.asarray(dec_w, dtype=np.float32)
    dec_b = np.asarray(dec_b, dtype=np.float32)
    cvg_b = np.asarray(cvg_b, dtype=np.float32)
    v = np.asarray(v, dtype=np.float32)
    # 'same' padding with kernel (1, H) on a single pixel: only the center
    # column of the conv weight is ever active.
    center = (H - 1) // 2
    w_eff = np.asarray(cvg_w[:, :, 0, center], dtype=np.float32)
    maskf = np.where(mask == 1, np.float32(0.0), np.float32(-1e38))
    # tiny linears precomputed host-side (0.2% of total FLOPs)
    covf = coverage @ w_eff.T + cvg_b                 # [B, L] cov_feat
    biasf = hidden @ dec_w.T + dec_b + attn_b         # [B, H] tanh bias

    nc = _get_program()
    in_maps = [
        _prep_core_inputs(c, enc, maskf, coverage, attn_w, v, covf, biasf)
        for c in range(NCORES)
    ]
    trace = os.environ.get("KERNEL_TRACE", "") == "1"
    res = run_bass_kernel_spmd(nc, in_maps, core_ids=list(range(NCORES)),
                               trace=trace)
    if trace and res.exec_time_ns is not None:
        _CACHE["exec_time_ns"] = res.exec_time_ns
        _CACHE["mean_exec_time_ns"] = res.mean_exec_time_ns
        _CACHE["trace"] = res.instructions_and_trace

    ctx = np.empty((B, H), np.float32)
    aw = np.empty((B, L), np.float32)
    ncov = np.empty((B, L), np.float32)
    for c in range(NCORES):
        r = res.results[c]
        s = slice(c * BLOC, (c + 1) * BLOC)
        aw[s] = r["aw_out"].astype(np.float32)
        ncov[s] = r["ncov_out"]
        ctx[s] = r["ctx_out"]
    return ctx, aw, ncov



# revision 9
# speedup vs baseline: 1.2011x; 1.2011x over previous
"""Trainium2 Bass kernel for nn_AttnCalc (coverage attention).

Contract: kernel(**inputs) takes FULL unsharded numpy inputs, distributes
batch-parallel across 8 NeuronCores, returns the full
(context_vector, attn_weights, new_coverage) tuple like the reference.

Math per batch b:
  enc_feat = enc[b] @ attn_w.T + attn_b          [L,H]
  dec_feat = dec_w @ hidden[b] + dec_b           [H]
  cov_feat = w_eff @ coverage[b] + cvg_b         [L]   (w_eff = cvg_w[:,:,0,(H-1)//2])
  feats    = tanh(enc_feat + dec_feat + cov_feat[:,None])
  scores   = feats @ v[b]  (masked, softmax over L) -> aw
  new_cov  = coverage[b] + aw
  context  = aw @ enc[b]                         [H]

Structure: the 8 local batches are processed as NG=2 groups of GB=4.
Within a group every PE matmul loop is batch-inner, so one stationary
weight (a 128x128 chunk of attn_w.T) serves 4 back-to-back matmuls and
LDWEIGHTS fully hides behind the previous matmul.  Scores for the 4
batches are accumulated into a single [4, L] PSUM tile via zero-padded
lhsT columns, so the whole softmax runs as [4, L] row-ops (4 DVE lanes
instead of 1).  aw is transposed on-chip with PE transpose-mode matmuls
(aw16 [4,128] -> [128,4] per l-chunk) instead of a DRAM round trip.
A dozen dummy matmuls at PE-queue head keep the PE HAM clock un-throttled
through the DMA preamble.

Mask compaction: only positions with attn_mask==1 contribute to the
outputs (scores at masked positions are -inf -> aw exactly 0, ncov equals
coverage).  The host gathers the ~B(512,1/2)~256 unmasked columns per
batch and pads to Lc=384 (11 sigma above the mean); the device kernel
only processes Lc columns.  If any batch ever exceeded Lc, a full
Lc=512 program is compiled and used instead (no compaction).

The target walrus build allows only ONE semaphore wait per TPB compute
instruction; _legalize_waits redistributes extra waits onto earlier
same-engine instructions (LDWEIGHTS/NOPs serve as landing spots).
"""

import sys
import os

sys.path.insert(0, "/opt/trn_rl_repo")

import numpy as np

import concourse.bass as bass
import concourse.tile as tile
from concourse import mybir
from concourse.bass_utils import run_bass_kernel_spmd
from concourse.tile_rust import add_dep_helper

B, L, H = 64, 512, 512
NCORES = 8
BLOC = B // NCORES          # batches per core
NG = 2                      # groups per core
GB = BLOC // NG             # batches per group (4)
P = 128                     # SBUF partitions
PC = H // P                 # 128-chunks along H
F32 = mybir.dt.float32
F16 = mybir.dt.float16
Tanh = mybir.ActivationFunctionType.Tanh
Exp = mybir.ActivationFunctionType.Exp
Copy = mybir.ActivationFunctionType.Copy

_CACHE = {}


def _build_program(Lc):
    LJ = Lc // P            # l-chunks for the context contraction
    nc = bass.Bass()

    # fp16 inputs
    encT = nc.declare_dram_parameter("encT", [NG, PC, P, GB, Lc], F16,
                                     isOutput=False)
    encN = nc.declare_dram_parameter("encN", [BLOC, P, LJ, H], F16,
                                     isOutput=False)
    attn_wPK = nc.declare_dram_parameter("attn_wPK", [PC, P, H], F16,
                                         isOutput=False)
    vS4z = nc.declare_dram_parameter("vS4z", [P, PC, NG, GB, GB], F16,
                                     isOutput=False)
    ident4 = nc.declare_dram_parameter("ident4", [GB, GB], F16,
                                       isOutput=False)
    covf16 = nc.declare_dram_parameter("covf16", [BLOC, Lc], F16,
                                       isOutput=False)
    # f32 inputs
    biasPE = nc.declare_dram_parameter("biasPE", [P, PC, BLOC], F32,
                                       isOutput=False)
    mask4 = nc.declare_dram_parameter("mask4", [GB, NG, Lc], F32,
                                      isOutput=False)
    covin4 = nc.declare_dram_parameter("covin4", [GB, NG, Lc], F32,
                                       isOutput=False)

    aw_out = nc.declare_dram_parameter("aw_out", [NG, GB, Lc], F16,
                                       isOutput=True)
    ncov_out = nc.declare_dram_parameter("ncov_out", [NG, GB, Lc], F32,
                                         isOutput=True)
    ctx_out = nc.declare_dram_parameter("ctx_out", [BLOC, H], F32,
                                        isOutput=True)

    def row3(dram2d, b=BLOC):
        # [b, l] dram -> [1, b, l] AP so rows live on partition 0
        return dram2d[:, :].rearrange("b l -> (b l)")[None].rearrange(
            "o (b l) -> o b l", b=b)

    with tile.TileContext(nc) as tc:
        with (
            tc.tile_pool(name="const", bufs=1) as const,
            tc.tile_pool(name="enc", bufs=8) as epool,
            tc.tile_pool(name="encn", bufs=8) as npool,
            tc.tile_pool(name="feat", bufs=8) as fpool,
            tc.tile_pool(name="aw", bufs=2) as apool,
            tc.tile_pool(name="eps", bufs=4,
                         space=bass.MemorySpace.PSUM) as ppool,
            tc.tile_pool(name="scps", bufs=1, space=bass.MemorySpace.PSUM) as scpool,
            tc.tile_pool(name="awps", bufs=1, space=bass.MemorySpace.PSUM) as tpool,
            tc.tile_pool(name="cxps", bufs=2, space=bass.MemorySpace.PSUM) as cxpool,
        ):
            # ---------------- constants ----------------
            # SP queue: wA k-chunks interleaved with eT group loads (below).
            wAk = []
            wAk_dma = []
            eTg = {}
            eTg_dma = {}
            for k in range(PC):
                t = const.tile([P, H], F16, name=f"wAk{k}")
                wAk.append(t)
                wAk_dma.append(nc.sync.dma_start(out=t, in_=attn_wPK[k]))
                te = epool.tile([P, GB, Lc], F16, tag="eT", name=f"eT0_{k}")
                eTg[(0, k)] = te
                eTg_dma[(0, k)] = nc.sync.dma_start(out=te, in_=encT[0, k])
            for k in range(PC):
                te = epool.tile([P, GB, Lc], F16, tag="eT", name=f"eT1_{k}")
                eTg[(1, k)] = te
                eTg_dma[(1, k)] = nc.sync.dma_start(out=te, in_=encT[1, k])

            # Pool queue: small constants in first-use order, then eN loads.
            id4 = const.tile([GB, GB], F16)
            nc.gpsimd.dma_start(out=id4, in_=ident4[:, :])
            cov16r = const.tile([1, BLOC, Lc], F16)
            cov16r_dma = nc.gpsimd.dma_start(out=cov16r, in_=row3(covf16))
            bias_sb = const.tile([P, PC, BLOC], F32)
            bias_dma = nc.gpsimd.dma_start(out=bias_sb, in_=biasPE[:, :, :])
            vz = const.tile([P, PC, NG, GB, GB], F16)
            vz_dma = nc.gpsimd.dma_start(out=vz, in_=vS4z[:, :, :, :, :])
            mb4 = const.tile([GB, NG, Lc], F32)
            mb4_dma = nc.gpsimd.dma_start(out=mb4, in_=mask4[:, :, :])
            cv4 = const.tile([GB, NG, Lc], F32)
            cv4_dma = nc.gpsimd.dma_start(out=cv4, in_=covin4[:, :, :])
            eN = {}
            for b in range(BLOC):
                t = npool.tile([P, LJ, H], F16, tag="eN", name=f"eN{b}")
                eN[b] = t
                nc.gpsimd.dma_start(out=t, in_=encN[b])

            # DVE: memsets + softmax row ops only.
            ones16_p = const.tile([1, P], F16)
            nc.vector.memset(ones16_p, 1.0)
            warm = const.tile([P, 512], F16)
            nc.vector.memset(warm, 0.0)

            scf = const.tile([GB, NG, Lc], F32)     # scores -> exp rows
            nmx = const.tile([GB, NG, 1], F32)
            se = const.tile([GB, NG, 1], F32)
            rse = const.tile([GB, NG, 1], F32)
            ctxr = const.tile([1, BLOC, H], F32)    # ctx rows staging

            # Early landing spots so first-use waits (const DMAs) can be
            # legalized onto dedicated instructions.  Pinned (order-only)
            # after the DMAs so they commit after the producers.
            for d in (cov16r_dma, bias_dma, vz_dma):
                for _ in range(2):
                    n0 = nc.scalar.nop(nofuse=True)
                    add_dep_helper(n0.ins, d.ins, sync=False,
                                   reason="landing spot")
            for d in (mb4_dma, cv4_dma):
                for _ in range(2):
                    n0 = nc.vector.nop(nofuse=True)
                    add_dep_helper(n0.ins, d.ins, sync=False,
                                   reason="landing spot")

            # ---------------- PE warmup ----------------
            # Dummy matmuls keep the PE busy (HAM un-throttle) while the
            # first data DMAs land.  They recycle the enc PSUM ring.
            for w in range(10):
                wps = ppool.tile([P, 512], F32, tag="encps", name=f"warm{w}")
                nc.tensor.matmul(wps, warm[:, 0:P], warm[:, :],
                                 start=True, stop=True)

            # ---------------- main pipeline ----------------
            ft = {}      # (g, b) -> feats tile [P, PC, Lc]
            awg = {}     # g -> aw fp16 [GB, Lc]
            awTs = {}    # g -> aw columns fp16 [P, LJ, GB]

            def emit_enc(g):
                for b in range(GB):
                    ft[(g, b)] = fpool.tile([P, PC, Lc], F16, tag="ft",
                                            name=f"ft{g}_{b}")
                for o in range(PC):
                    ps = [ppool.tile([P, Lc], F32, tag="encps",
                                     name=f"ps{g}_{o}_{b}") for b in range(GB)]
                    for k in range(PC):
                        for b in range(GB):
                            nc.tensor.matmul(ps[b],
                                             wAk[k][:, o * P:(o + 1) * P],
                                             eTg[(g, k)][:, b, :],
                                             start=(k == 0), stop=False)
                    for b in range(GB):
                        nc.tensor.matmul(ps[b], ones16_p[:, :],
                                         cov16r[0:1, g * GB + b, :],
                                         start=False, stop=True)
                    for b in range(GB):
                        nc.scalar.activation(
                            out=ft[(g, b)][:, o, :], in_=ps[b], func=Tanh,
                            bias=bias_sb[:, o, g * GB + b:g * GB + b + 1],
                            scale=1.0)
                    yield o

            def emit_scores_softmax(g):
                sc_ps = scpool.tile([GB, Lc], F32, tag="sc")
                n = GB * PC
                i = 0
                for b in range(GB):
                    for k in range(PC):
                        nc.tensor.matmul(sc_ps, vz[:, k, g, :, b],
                                         ft[(g, b)][:, k, :],
                                         start=(i == 0), stop=(i == n - 1),
                                         skip_group_check=True)
                        i += 1
                for b in range(GB):
                    ft.pop((g, b))
                scr = scf[:, g, :]
                madd = nc.vector.tensor_add(scr, sc_ps, mb4[:, g, :])
                nc.vector.tensor_reduce(out=nmx[:, g, :], in_=scr,
                                        axis=mybir.AxisListType.X,
                                        op=mybir.AluOpType.max, negate=True)
                nc.scalar.activation(out=scr, in_=scr, func=Exp,
                                     bias=nmx[:, g, :], scale=1.0,
                                     accum_out=se[:, g, :])
                nc.vector.reciprocal(rse[:, g, :], se[:, g, :])
                aw16 = apool.tile([GB, Lc], F16, tag="aw16", name=f"aw16_{g}")
                awg[g] = aw16
                nc.vector.tensor_scalar_mul(aw16, scr, rse[:, g, :])
                # new coverage + outputs (Pool queue)
                ncadd = nc.vector.tensor_add(cv4[:, g, :], cv4[:, g, :], aw16)
                gp = [nc.gpsimd.nop(nofuse=True) for _ in range(3)]
                add_dep_helper(gp[0].ins, ncadd.ins, sync=False,
                               reason="order outputs after softmax")
                for j in range(1, 3):
                    add_dep_helper(gp[j].ins, gp[j - 1].ins, sync=False,
                                   reason="landing chain")
                aw_dma = nc.gpsimd.dma_start(out=aw_out[g], in_=aw16)
                add_dep_helper(aw_dma.ins, gp[2].ins, sync=False,
                               reason="after landing nops")
                nc.gpsimd.dma_start(out=ncov_out[g], in_=cv4[:, g, :])

            def emit_ctx(g):
                # aw transpose: [GB, 128] chunks -> [128, GB] columns
                awT_ps = tpool.tile([P, LJ * GB], F16, tag="awt",
                                    name=f"awT{g}")
                for j in range(LJ):
                    nc.tensor.transpose(awT_ps[:, j * GB:(j + 1) * GB],
                                        awg[g][0:GB, j * P:(j + 1) * P],
                                        id4[0:GB, 0:GB])
                aw4 = apool.tile([P, LJ, GB], F16, tag="aw4", name=f"aw4_{g}")
                awTs[g] = aw4
                nc.scalar.activation(
                    out=aw4[:, :, :],
                    in_=awT_ps[:, :].rearrange("p (j g) -> p j g", j=LJ),
                    func=Copy)
                last_dma = None
                for b in range(GB):
                    cx = cxpool.tile([1, H], F32, tag="cx",
                                     name=f"cx{g}_{b}")
                    for j in range(LJ):
                        nc.tensor.matmul(cx, aw4[:, j, b:b + 1],
                                         eN[g * GB + b][:, j, :],
                                         start=(j == 0), stop=(j == LJ - 1))
                    nc.scalar.copy(ctxr[0:1, g * GB + b, :], cx)
                gp = [nc.gpsimd.nop(nofuse=True) for _ in range(2)]
                for j in range(1, 2):
                    add_dep_helper(gp[j].ins, gp[j - 1].ins, sync=False,
                                   reason="landing chain")
                cx_dma = nc.gpsimd.dma_start(
                    out=ctx_out[g * GB:(g + 1) * GB, :].rearrange(
                        "b h -> (b h)")[None].rearrange("o (b h) -> o b h",
                                                        b=GB),
                    in_=ctxr[0:1, g * GB:(g + 1) * GB, :])
                add_dep_helper(cx_dma.ins, gp[1].ins, sync=False,
                               reason="after landing nops")
                return cx_dma

            # group 0 enc + scores
            for _o in emit_enc(0):
                pass
            emit_scores_softmax(0)
            # group 1 enc, with group-0 ctx interleaved after o==0
            gen1 = emit_enc(1)
            next(gen1)
            cx_dma = emit_ctx(0)
            for _o in gen1:
                pass
            emit_scores_softmax(1)
            cx_dma = emit_ctx(1)

            # tail landing slots for the kernel-tail drain waits
            tail = nc.gpsimd.nop(nofuse=True)
            add_dep_helper(tail.ins, cx_dma.ins, sync=False, reason="tail")
            for _ in range(24):
                n2 = nc.sync.nop(nofuse=True)
                add_dep_helper(n2.ins, tail.ins, sync=False, reason="tail")
                tail = n2
            gtail = tail
            for _ in range(6):
                n2 = nc.gpsimd.nop(nofuse=True)
                add_dep_helper(n2.ins, gtail.ins, sync=False, reason="tail")
                gtail = n2

    _legalize_waits(nc)
    return nc


# The nix walrus build (setupSyncWait) accepts only ONE sync wait per TPB
# instruction (compute and DMA alike).  Tile can emit several.  Because the
# committed instruction order is a topological order of the dependency
# graph, a wait whose producing semaphore update completes at block index p
# can be safely carried by ANY same-engine instruction at index > p that
# precedes the original carrier: engines execute in order, so the original
# instruction still starts after the wait is satisfied, and the producer
# (committed before the new carrier) cannot depend on it -- no deadlock.
# Assign waits to instructions as an interval matching problem.
def _legalize_waits(nc):
    import concourse.mybir as _mb

    fn = nc.m.functions[0]
    stuck = []
    NO_LANDING = ("InstISA", "InstEventSemaphore", "InstUnconditionalBranch",
                  "InstCall", "InstRegisterMove", "InstHalt")
    insts = []
    for blk in fn.blocks:
        insts.extend(blk.instructions)

    sem_hist = {}
    cum = {}
    streams = {}
    for i, inst in enumerate(insts):
        si = inst.sync_info
        if si is not None:
            for u in si.on_update:
                cum[u.id] = cum.get(u.id, 0) + u.update_value
                sem_hist.setdefault(u.id, []).append((i, cum[u.id]))
        streams.setdefault(inst.engine, []).append(i)

    def producer_idx(w):
        hist = sem_hist.get(w.id)
        if hist is None:
            return None            # unknown semaphore: not movable
        for i, v in hist:
            if v >= w.wait_value:
                return i
        return None

    for eng, stream in streams.items():
        movable_spos = []
        pinned = {}                # spos -> unmovable waits
        waits = []                 # (carrier_spos, producer_bidx, wait)
        has_multi = False
        pos_of = {i: spos for spos, i in enumerate(stream)}
        eng_name = str(eng).split(".")[-1]
        for spos, i in enumerate(stream):
            inst = insts[i]
            si = inst.sync_info
            ws = list(si.on_wait) if si is not None else []
            if len(ws) > 1:
                has_multi = True
            # Waits on this engine's own execution-counter semaphore whose
            # producing (non-DMA) instruction ran >=8 instructions earlier
            # on this engine are redundant: engine-counter updates fire in
            # engine order, and 8 instructions is far beyond the pipeline
            # write-drain window.  DMA-completion sems fire asynchronously
            # and are never dropped.
            def _redundant(w):
                if w.ant_name.split("_")[0] != eng_name:
                    return False
                p = producer_idx(w)
                return (p is not None and p in pos_of
                        and insts[p].__class__.__name__ != "InstDMACopy"
                        and spos - pos_of[p] >= 8)
            nws = [w for w in ws if not _redundant(w)]
            if len(nws) != len(ws):
                has_multi = True
            ws = nws

            def mov(w):
                if w.wait_reg is not None or w.wait_value <= 0:
                    return False
                p = producer_idx(w)
                return p is not None and p < i
            special = inst.__class__.__name__ in NO_LANDING
            unmov = [w for w in ws if special or not mov(w)]
            if unmov:
                pinned[spos] = unmov
            elif not special:
                movable_spos.append(spos)
            if special:
                continue
            best = {}
            for w in ws:
                if not mov(w):
                    continue
                if w.id not in best or w.wait_value > best[w.id].wait_value:
                    best[w.id] = w
            for w in best.values():
                waits.append((spos, producer_idx(w), w))
        if not has_multi:
            continue
        bidx_of = {spos: stream[spos] for spos in range(len(stream))}
        free = sorted(movable_spos)
        assign = {}
        for carrier, pbidx, w in sorted(waits, key=lambda t: (t[0], -t[1])):
            chosen = None
            for spos in reversed(free):
                if spos > carrier:
                    continue
                if bidx_of[spos] <= pbidx:
                    break
                chosen = spos
                break
            if chosen is None:
                stuck.append((insts[stream[carrier]].name,
                              insts[stream[carrier]].__class__.__name__,
                              w.ant_name, w.wait_value))
                continue
            free.remove(chosen)
            assign.setdefault(chosen, []).append(w)
        for spos in range(len(stream)):
            inst = insts[stream[spos]]
            si = inst.sync_info
            ups = list(si.on_update) if si is not None else []
            new_w = pinned.get(spos, []) + assign.get(spos, [])
            if si is None and not new_w:
                continue
            inst.sync_info = _mb.SyncInfo(on_wait=new_w, on_update=ups)
    if stuck:
        raise RuntimeError(f"wait legalization failed: {stuck[:8]}")


def _get_program(Lc):
    key = ("nc", Lc)
    if key not in _CACHE:
        _CACHE[key] = _build_program(Lc)
    return _CACHE[key]


def _prep_core_inputs(c, Lc, idx_all, enc, maskc_f, coverage, attn_w, v,
                      covf, biasf):
    s0 = c * BLOC
    LJ = Lc // P
    encTa = np.zeros((NG, PC, P, GB, Lc), np.float16)
    encNa = np.zeros((BLOC, P, LJ, H), np.float16)
    covc = np.zeros((BLOC, Lc), np.float16)
    m4 = np.full((GB, NG, Lc), -1e38, np.float32)
    cv4 = np.zeros((GB, NG, Lc), np.float32)
    vz = np.zeros((P, PC, NG, GB, GB), np.float16)
    for i in range(BLOC):
        gb = s0 + i
        idx = idx_all[gb]
        n = len(idx)
        g, bi = divmod(i, GB)
        enc_c = enc[gb, idx].astype(np.float16)          # [n, H]
        # encT[g, k, p, b, l'] = enc_c[l', 128k+p]
        eT = enc_c.T.reshape(PC, P, n)                    # [k, p, n]
        encTa[g, :, :, bi, :n] = eT
        # encN[i, p, j, h] = enc_c[128j+p, h]
        pad = np.zeros((LJ * P - n, H), np.float16)
        encNa[i] = np.concatenate([enc_c, pad]).reshape(LJ, P, H).transpose(1, 0, 2)
        covc[i, :n] = covf[gb, idx]
        m4[bi, g, :n] = 0.0
        cv4[bi, g, :n] = coverage[gb, idx]
        # vS4z[p, k, g, m, b] = v[gb, 128k+p] iff m == b
        vz[:, :, g, bi, bi] = v[gb].reshape(PC, P).T
    return {
        "encT": encTa,
        "encN": encNa,
        "attn_wPK": np.ascontiguousarray(
            attn_w.T.astype(np.float16).reshape(PC, P, H)),
        "vS4z": vz,
        "ident4": np.eye(GB, dtype=np.float16),
        "covf16": covc,
        "biasPE": np.ascontiguousarray(
            biasf[s0:s0 + BLOC].T.reshape(PC, P, BLOC).transpose(1, 0, 2)),
        "mask4": m4,
        "covin4": cv4,
    }


def kernel(encoder_outputs, attn_mask, hidden, coverage,
           attn_w, attn_b, dec_w, dec_b, cvg_w, cvg_b, v):
    enc = np.asarray(encoder_outputs, dtype=np.float32)
    mask = np.asarray(attn_mask)
    hidden = np.asarray(hidden, dtype=np.float32)
    coverage = np.asarray(coverage, dtype=np.float32)
    attn_w = np.asarray(attn_w, dtype=np.float32)
    attn_b = np.asarray(attn_b, dtype=np.float32)
    dec_w = np.asarray(dec_w, dtype=np.float32)
    dec_b = np.asarray(dec_b, dtype=np.float32)
    cvg_b = np.asarray(cvg_b, dtype=np.float32)
    v = np.asarray(v, dtype=np.float32)
    # 'same' padding with kernel (1, H) on a single pixel: only the center
    # column of the conv weight is ever active.
    center = (H - 1) // 2
    w_eff = np.asarray(cvg_w[:, :, 0, center], dtype=np.float32)
    # tiny linears precomputed host-side (0.2% of total FLOPs)
    covf = coverage @ w_eff.T + cvg_b                 # [B, L] cov_feat
    biasf = hidden @ dec_w.T + dec_b + attn_b         # [B, H] tanh bias

    # mask compaction: keep only mask==1 columns, pad to Lc
    idx_all = [np.nonzero(mask[b] == 1)[0] for b in range(B)]
    max_n = max(len(ix) for ix in idx_all)
    Lc = 384
    if max_n > Lc:
        Lc = L
        idx_all = [np.arange(L) for _ in range(B)]

    nc = _get_program(Lc)
    in_maps = [
        _prep_core_inputs(c, Lc, idx_all, enc, mask, coverage, attn_w, v,
                          covf, biasf)
        for c in range(NCORES)
    ]
    trace = os.environ.get("KERNEL_TRACE", "") == "1"
    res = run_bass_kernel_spmd(nc, in_maps, core_ids=list(range(NCORES)),
                               trace=trace)
    if trace and res.exec_time_ns is not None:
        _CACHE["exec_time_ns"] = res.exec_time_ns
        _CACHE["mean_exec_time_ns"] = res.mean_exec_time_ns
        _CACHE["trace"] = res.instructions_and_trace

    ctx = np.empty((B, H), np.float32)
    aw = np.zeros((B, L), np.float32)
    ncov = coverage.copy()
    for c in range(NCORES):
        r = res.results[c]
        aw_c = r["aw_out"].reshape(NG, GB, Lc)
        ncov_c = r["ncov_out"].reshape(NG, GB, Lc)
        ctx[c * BLOC:(c + 1) * BLOC] = r["ctx_out"]
        for i in range(BLOC):
            gb = c * BLOC + i
            idx = idx_all[gb]
            n = len(idx)
            g, bi = divmod(i, GB)
            aw[gb, idx] = aw_c[g, bi, :n].astype(np.float32)
            ncov[gb, idx] = ncov_c[g, bi, :n]
    return ctx, aw, ncov


# revision 11
# speedup vs baseline: 1.3211x; 1.0999x over previous
"""Trainium2 Bass kernel for nn_AttnCalc (coverage attention).

Contract: kernel(**inputs) takes FULL unsharded numpy inputs, distributes
batch-parallel across 8 NeuronCores, returns the full
(context_vector, attn_weights, new_coverage) tuple like the reference.

Math per batch b:
  enc_feat = enc[b] @ attn_w.T + attn_b          [L,H]
  dec_feat = dec_w @ hidden[b] + dec_b           [H]
  cov_feat = w_eff @ coverage[b] + cvg_b         [L]   (w_eff = cvg_w[:,:,0,(H-1)//2])
  feats    = tanh(enc_feat + dec_feat + cov_feat[:,None])
  scores   = feats @ v[b]  (masked, softmax over L) -> aw
  new_cov  = coverage[b] + aw
  context  = aw @ enc[b]                         [H]

Structure: the 8 local batches are processed as NG=2 groups of GB=4.
Within a group every PE matmul loop is batch-inner, so one stationary
weight (a 128x128 chunk of attn_w.T) serves 4 back-to-back matmuls and
LDWEIGHTS fully hides behind the previous matmul.  Scores for the 4
batches are accumulated into a single [4, L] PSUM tile via zero-padded
lhsT columns, so the whole softmax runs as [4, L] row-ops (4 DVE lanes
instead of 1).  aw is transposed on-chip with PE transpose-mode matmuls
(aw16 [4,128] -> [128,4] per l-chunk) instead of a DRAM round trip.
A dozen dummy matmuls at PE-queue head keep the PE HAM clock un-throttled
through the DMA preamble.

Mask compaction: only positions with attn_mask==1 contribute to the
outputs (scores at masked positions are -inf -> aw exactly 0, ncov equals
coverage).  The host gathers the ~B(512,1/2)~256 unmasked columns per
batch and pads to Lc=384 (11 sigma above the mean); the device kernel
only processes Lc columns.  If any batch ever exceeded Lc, a full
Lc=512 program is compiled and used instead (no compaction).

The target walrus build allows only ONE semaphore wait per TPB compute
instruction; _legalize_waits redistributes extra waits onto earlier
same-engine instructions (LDWEIGHTS/NOPs serve as landing spots).
"""

import sys
import os

sys.path.insert(0, "/opt/trn_rl_repo")

import numpy as np

import concourse.bass as bass
import concourse.tile as tile
from concourse import mybir
from concourse.bass_utils import run_bass_kernel_spmd
from concourse.tile_rust import add_dep_helper

B, L, H = 64, 512, 512
NCORES = 8
BLOC = B // NCORES          # batches per core
NG = 2                      # groups per core
GB = BLOC // NG             # batches per group (4)
P = 128                     # SBUF partitions
PC = H // P                 # 128-chunks along H
F32 = mybir.dt.float32
F16 = mybir.dt.float16
Tanh = mybir.ActivationFunctionType.Tanh
Exp = mybir.ActivationFunctionType.Exp
Copy = mybir.ActivationFunctionType.Copy

_CACHE = {}


def _build_program(Lc):
    LJ = Lc // P            # l-chunks for the context contraction
    nc = bass.Bass()

    # fp16 inputs
    encT = nc.declare_dram_parameter("encT", [NG, PC, P, GB, Lc], F16,
                                     isOutput=False)
    encN = nc.declare_dram_parameter("encN", [BLOC, P, LJ, H], F16,
                                     isOutput=False)
    attn_wPK = nc.declare_dram_parameter("attn_wPK", [PC, P, H], F16,
                                         isOutput=False)
    vS4z = nc.declare_dram_parameter("vS4z", [P, PC, NG, GB, GB], F16,
                                     isOutput=False)
    ident4 = nc.declare_dram_parameter("ident4", [GB, GB], F16,
                                       isOutput=False)
    covB = nc.declare_dram_parameter("covB", [P, BLOC, Lc], F16,
                                     isOutput=False)
    # f32 inputs
    biasPE = nc.declare_dram_parameter("biasPE", [P, PC, BLOC], F32,
                                       isOutput=False)
    mask4 = nc.declare_dram_parameter("mask4", [GB, NG, Lc], F32,
                                      isOutput=False)

    aw_out = nc.declare_dram_parameter("aw_out", [NG, GB, Lc], F16,
                                       isOutput=True)
    se_out = nc.declare_dram_parameter("se_out", [NG, GB, 1], F32,
                                       isOutput=True)
    ctx_out = nc.declare_dram_parameter("ctx_out", [BLOC, H], F32,
                                        isOutput=True)

    def row3(dram2d, b=BLOC):
        # [b, l] dram -> [1, b, l] AP so rows live on partition 0
        return dram2d[:, :].rearrange("b l -> (b l)")[None].rearrange(
            "o (b l) -> o b l", b=b)

    with tile.TileContext(nc) as tc:
        with (
            tc.tile_pool(name="const", bufs=1) as const,
            tc.tile_pool(name="enc", bufs=8) as epool,
            tc.tile_pool(name="encn", bufs=8) as npool,
            tc.tile_pool(name="feat", bufs=8) as fpool,
            tc.tile_pool(name="aw", bufs=2) as apool,
            tc.tile_pool(name="eps", bufs=4,
                         space=bass.MemorySpace.PSUM) as ppool,
            tc.tile_pool(name="scps", bufs=1, space=bass.MemorySpace.PSUM) as scpool,
            tc.tile_pool(name="awps", bufs=1, space=bass.MemorySpace.PSUM) as tpool,
            tc.tile_pool(name="cxps", bufs=2, space=bass.MemorySpace.PSUM) as cxpool,
        ):
            # ---------------- constants ----------------
            # SP queue: wA k-chunks interleaved with eT group loads (below).
            wAk = []
            wAk_dma = []
            eTg = {}
            eTg_dma = {}
            for k in range(PC):
                t = const.tile([P, H], F16, name=f"wAk{k}")
                wAk.append(t)
                wAk_dma.append(nc.sync.dma_start(out=t, in_=attn_wPK[k]))
                te = epool.tile([P, GB, Lc], F16, tag="eT", name=f"eT0_{k}")
                eTg[(0, k)] = te
                eTg_dma[(0, k)] = nc.sync.dma_start(out=te, in_=encT[0, k])
            for k in range(PC):
                te = epool.tile([P, GB, Lc], F16, tag="eT", name=f"eT1_{k}")
                eTg[(1, k)] = te
                eTg_dma[(1, k)] = nc.sync.dma_start(out=te, in_=encT[1, k])

            # Pool queue: small constants in first-use order, then eN loads.
            id4 = const.tile([GB, GB], F16)
            nc.gpsimd.dma_start(out=id4, in_=ident4[:, :])
            covBs = const.tile([P, BLOC, Lc], F16)
            cov16r_dma = nc.gpsimd.dma_start(out=covBs, in_=covB[:, :, :])
            bias_sb = const.tile([P, PC, BLOC], F32)
            bias_dma = nc.gpsimd.dma_start(out=bias_sb, in_=biasPE[:, :, :])
            vz = const.tile([P, PC, NG, GB, GB], F16)
            vz_dma = nc.gpsimd.dma_start(out=vz, in_=vS4z[:, :, :, :, :])
            mb4 = const.tile([GB, NG, Lc], F32)
            mb4_dma = nc.gpsimd.dma_start(out=mb4, in_=mask4[:, :, :])
            # eN loads chained behind the last eT load so they cannot steal
            # DMA bandwidth from the PE-critical eT stream.
            eN = {}
            prev_bulk = eTg_dma[(1, PC - 1)]
            for b in range(BLOC):
                t = npool.tile([P, LJ, H], F16, tag="eN", name=f"eN{b}")
                eN[b] = t
                d = nc.gpsimd.dma_start(out=t, in_=encN[b])
                if b == 0:
                    add_dep_helper(d.ins, prev_bulk.ins, sync=True,
                                   reason="eN after eT stream")

            # DVE: memsets + softmax row ops only.
            ones128 = const.tile([P, P], F16)
            nc.vector.memset(ones128, 1.0)
            warm = const.tile([P, 512], F16)
            nc.vector.memset(warm, 0.0)

            scf = const.tile([GB, NG, Lc], F32)     # scores -> exp rows
            nmx = const.tile([GB, NG, 1], F32)
            se = const.tile([GB, NG, 1], F32)
            rse = const.tile([GB, NG, 1], F32)
            ctxr = const.tile([1, BLOC, H], F32)    # ctx rows staging

            # Early landing spots so first-use waits (const DMAs) can be
            # legalized onto dedicated instructions.  Pinned (order-only)
            # after the DMAs so they commit after the producers.
            for d in (cov16r_dma, bias_dma, vz_dma):
                for _ in range(2):
                    n0 = nc.scalar.nop(nofuse=True)
                    add_dep_helper(n0.ins, d.ins, sync=False,
                                   reason="landing spot")
            for d in (mb4_dma, mb4_dma):
                for _ in range(2):
                    n0 = nc.vector.nop(nofuse=True)
                    add_dep_helper(n0.ins, d.ins, sync=False,
                                   reason="landing spot")

            # ---------------- PE warmup ----------------
            # Dummy matmuls keep the PE busy (HAM un-throttle) while the
            # first data DMAs land.  They recycle the enc PSUM ring.
            for w in range(10):
                wps = ppool.tile([P, 512], F32, tag="encps", name=f"warm{w}")
                nc.tensor.matmul(wps, warm[:, 0:P], warm[:, :],
                                 start=True, stop=True)

            # ---------------- main pipeline ----------------
            ft = {}      # (g, b) -> feats tile [P, PC, Lc]
            awg = {}     # g -> aw fp16 [GB, Lc]
            awTs = {}    # g -> aw columns fp16 [P, LJ, GB]

            def emit_enc(g):
                for b in range(GB):
                    ft[(g, b)] = fpool.tile([P, PC, Lc], F16, tag="ft",
                                            name=f"ft{g}_{b}")
                for o in range(PC):
                    ps = [ppool.tile([P, Lc], F32, tag="encps",
                                     name=f"ps{g}_{o}_{b}") for b in range(GB)]
                    for k in range(PC):
                        for b in range(GB):
                            nc.tensor.matmul(ps[b],
                                             wAk[k][:, o * P:(o + 1) * P],
                                             eTg[(g, k)][:, b, :],
                                             start=(k == 0), stop=False)
                    for b in range(GB):
                        nc.tensor.matmul(ps[b], ones128[:, :],
                                         covBs[:, g * GB + b, :],
                                         start=False, stop=True)
                    for b in range(GB):
                        nc.scalar.activation(
                            out=ft[(g, b)][:, o, :], in_=ps[b], func=Tanh,
                            bias=bias_sb[:, o, g * GB + b:g * GB + b + 1],
                            scale=1.0)
                    yield o

            def emit_scores_softmax(g):
                sc_ps = scpool.tile([GB, Lc], F32, tag="sc")
                n = GB * PC
                i = 0
                for b in range(GB):
                    for k in range(PC):
                        nc.tensor.matmul(sc_ps, vz[:, k, g, :, b],
                                         ft[(g, b)][:, k, :],
                                         start=(i == 0), stop=(i == n - 1),
                                         skip_group_check=True)
                        i += 1
                for b in range(GB):
                    ft.pop((g, b))
                scr = scf[:, g, :]
                madd = nc.vector.tensor_add(scr, sc_ps, mb4[:, g, :])
                nc.vector.tensor_reduce(out=nmx[:, g, :], in_=scr,
                                        axis=mybir.AxisListType.X,
                                        op=mybir.AluOpType.max, negate=True)
                e16 = apool.tile([GB, Lc], F16, tag="aw16", name=f"e16_{g}")
                awg[g] = e16
                expi = nc.scalar.activation(out=e16, in_=scr, func=Exp,
                                            bias=nmx[:, g, :], scale=1.0,
                                            accum_out=se[:, g, :])
                # unnormalized exp rows + row sums out; host normalizes
                gp = [nc.gpsimd.nop(nofuse=True) for _ in range(3)]
                add_dep_helper(gp[0].ins, expi.ins, sync=False,
                               reason="order outputs after softmax")
                for j in range(1, 3):
                    add_dep_helper(gp[j].ins, gp[j - 1].ins, sync=False,
                                   reason="landing chain")
                aw_dma = nc.gpsimd.dma_start(out=aw_out[g], in_=e16)
                add_dep_helper(aw_dma.ins, gp[2].ins, sync=False,
                               reason="after landing nops")
                nc.gpsimd.dma_start(out=se_out[g], in_=se[:, g, :])

            def emit_ctx(g):
                # aw transpose: [GB, 128] chunks -> [128, GB] columns
                awT_ps = tpool.tile([P, LJ * GB], F16, tag="awt",
                                    name=f"awT{g}")
                for j in range(LJ):
                    nc.tensor.transpose(awT_ps[:, j * GB:(j + 1) * GB],
                                        awg[g][0:GB, j * P:(j + 1) * P],
                                        id4[0:GB, 0:GB])
                aw4 = apool.tile([P, LJ, GB], F16, tag="aw4", name=f"aw4_{g}")
                awTs[g] = aw4
                nc.scalar.activation(
                    out=aw4[:, :, :],
                    in_=awT_ps[:, :].rearrange("p (j g) -> p j g", j=LJ),
                    func=Copy)
                last_dma = None
                for b in range(GB):
                    cx = cxpool.tile([1, H], F32, tag="cx",
                                     name=f"cx{g}_{b}")
                    for j in range(LJ):
                        nc.tensor.matmul(cx, aw4[:, j, b:b + 1],
                                         eN[g * GB + b][:, j, :],
                                         start=(j == 0), stop=(j == LJ - 1))
                    nc.scalar.copy(ctxr[0:1, g * GB + b, :], cx)
                gp = [nc.gpsimd.nop(nofuse=True) for _ in range(2)]
                for j in range(1, 2):
                    add_dep_helper(gp[j].ins, gp[j - 1].ins, sync=False,
                                   reason="landing chain")
                cx_dma = nc.gpsimd.dma_start(
                    out=ctx_out[g * GB:(g + 1) * GB, :].rearrange(
                        "b h -> (b h)")[None].rearrange("o (b h) -> o b h",
                                                        b=GB),
                    in_=ctxr[0:1, g * GB:(g + 1) * GB, :])
                add_dep_helper(cx_dma.ins, gp[1].ins, sync=False,
                               reason="after landing nops")
                return cx_dma

            # group 0 enc + scores
            for _o in emit_enc(0):
                pass
            emit_scores_softmax(0)
            # group 1 enc, with group-0 ctx interleaved after o==0
            gen1 = emit_enc(1)
            next(gen1)
            cx_dma = emit_ctx(0)
            for _o in gen1:
                pass
            emit_scores_softmax(1)
            cx_dma = emit_ctx(1)

            # tail landing slots for the kernel-tail drain waits
            tail = nc.gpsimd.nop(nofuse=True)
            add_dep_helper(tail.ins, cx_dma.ins, sync=False, reason="tail")
            for _ in range(24):
                n2 = nc.sync.nop(nofuse=True)
                add_dep_helper(n2.ins, tail.ins, sync=False, reason="tail")
                tail = n2
            gtail = tail
            for _ in range(6):
                n2 = nc.gpsimd.nop(nofuse=True)
                add_dep_helper(n2.ins, gtail.ins, sync=False, reason="tail")
                gtail = n2

    _legalize_waits(nc)
    return nc


# The nix walrus build (setupSyncWait) accepts only ONE sync wait per TPB
# instruction (compute and DMA alike).  Tile can emit several.  Because the
# committed instruction order is a topological order of the dependency
# graph, a wait whose producing semaphore update completes at block index p
# can be safely carried by ANY same-engine instruction at index > p that
# precedes the original carrier: engines execute in order, so the original
# instruction still starts after the wait is satisfied, and the producer
# (committed before the new carrier) cannot depend on it -- no deadlock.
# Assign waits to instructions as an interval matching problem.
def _legalize_waits(nc):
    import concourse.mybir as _mb

    fn = nc.m.functions[0]
    stuck = []
    NO_LANDING = ("InstISA", "InstEventSemaphore", "InstUnconditionalBranch",
                  "InstCall", "InstRegisterMove", "InstHalt")
    insts = []
    for blk in fn.blocks:
        insts.extend(blk.instructions)

    sem_hist = {}
    cum = {}
    streams = {}
    for i, inst in enumerate(insts):
        si = inst.sync_info
        if si is not None:
            for u in si.on_update:
                cum[u.id] = cum.get(u.id, 0) + u.update_value
                sem_hist.setdefault(u.id, []).append((i, cum[u.id]))
        streams.setdefault(inst.engine, []).append(i)

    def producer_idx(w):
        hist = sem_hist.get(w.id)
        if hist is None:
            return None            # unknown semaphore: not movable
        for i, v in hist:
            if v >= w.wait_value:
                return i
        return None

    for eng, stream in streams.items():
        movable_spos = []
        pinned = {}                # spos -> unmovable waits
        waits = []                 # (carrier_spos, producer_bidx, wait)
        has_multi = False
        pos_of = {i: spos for spos, i in enumerate(stream)}
        eng_name = str(eng).split(".")[-1]
        for spos, i in enumerate(stream):
            inst = insts[i]
            si = inst.sync_info
            ws = list(si.on_wait) if si is not None else []
            if len(ws) > 1:
                has_multi = True
            # Waits on this engine's own execution-counter semaphore whose
            # producing (non-DMA) instruction ran >=8 instructions earlier
            # on this engine are redundant: engine-counter updates fire in
            # engine order, and 8 instructions is far beyond the pipeline
            # write-drain window.  DMA-completion sems fire asynchronously
            # and are never dropped.
            def _redundant(w):
                if w.ant_name.split("_")[0] != eng_name:
                    return False
                p = producer_idx(w)
                return (p is not None and p in pos_of
                        and insts[p].__class__.__name__ != "InstDMACopy"
                        and spos - pos_of[p] >= 8)
            nws = [w for w in ws if not _redundant(w)]
            if len(nws) != len(ws):
                has_multi = True
            ws = nws

            def mov(w):
                if w.wait_reg is not None or w.wait_value <= 0:
                    return False
                p = producer_idx(w)
                return p is not None and p < i
            special = inst.__class__.__name__ in NO_LANDING
            unmov = [w for w in ws if special or not mov(w)]
            if unmov:
                pinned[spos] = unmov
            elif not special:
                movable_spos.append(spos)
            if special:
                continue
            best = {}
            for w in ws:
                if not mov(w):
                    continue
                if w.id not in best or w.wait_value > best[w.id].wait_value:
                    best[w.id] = w
            for w in best.values():
                waits.append((spos, producer_idx(w), w))
        if not has_multi:
            continue
        bidx_of = {spos: stream[spos] for spos in range(len(stream))}
        free = sorted(movable_spos)
        assign = {}
        for carrier, pbidx, w in sorted(waits, key=lambda t: (t[0], -t[1])):
            chosen = None
            for spos in reversed(free):
                if spos > carrier:
                    continue
                if bidx_of[spos] <= pbidx:
                    break
                chosen = spos
                break
            if chosen is None:
                stuck.append((insts[stream[carrier]].name,
                              insts[stream[carrier]].__class__.__name__,
                              w.ant_name, w.wait_value))
                continue
            free.remove(chosen)
            assign.setdefault(chosen, []).append(w)
        for spos in range(len(stream)):
            inst = insts[stream[spos]]
            si = inst.sync_info
            ups = list(si.on_update) if si is not None else []
            new_w = pinned.get(spos, []) + assign.get(spos, [])
            if si is None and not new_w:
                continue
            inst.sync_info = _mb.SyncInfo(on_wait=new_w, on_update=ups)
    if stuck:
        raise RuntimeError(f"wait legalization failed: {stuck[:8]}")


def _get_program(Lc):
    key = ("nc", Lc)
    if key not in _CACHE:
        _CACHE[key] = _build_program(Lc)
    return _CACHE[key]


def _prep_core_inputs(c, Lc, idx_all, enc, maskc_f, coverage, attn_w, v,
                      covf, biasf):
    s0 = c * BLOC
    LJ = Lc // P
    encTa = np.zeros((NG, PC, P, GB, Lc), np.float16)
    encNa = np.zeros((BLOC, P, LJ, H), np.float16)
    covc = np.zeros((BLOC, Lc), np.float32)
    m4 = np.full((GB, NG, Lc), -1e38, np.float32)
    vz = np.zeros((P, PC, NG, GB, GB), np.float16)
    for i in range(BLOC):
        gb = s0 + i
        idx = idx_all[gb]
        n = len(idx)
        g, bi = divmod(i, GB)
        enc_c = enc[gb, idx].astype(np.float16)          # [n, H]
        # encT[g, k, p, b, l'] = enc_c[l', 128k+p]
        eT = enc_c.T.reshape(PC, P, n)                    # [k, p, n]
        encTa[g, :, :, bi, :n] = eT
        # encN[i, p, j, h] = enc_c[128j+p, h]
        pad = np.zeros((LJ * P - n, H), np.float16)
        encNa[i] = np.concatenate([enc_c, pad]).reshape(LJ, P, H).transpose(1, 0, 2)
        covc[i, :n] = covf[gb, idx]
        m4[bi, g, :n] = 0.0
        # vS4z[p, k, g, m, b] = v[gb, 128k+p] iff m == b
        vz[:, :, g, bi, bi] = v[gb].reshape(PC, P).T
    return {
        "encT": encTa,
        "encN": encNa,
        "attn_wPK": np.ascontiguousarray(
            attn_w.T.astype(np.float16).reshape(PC, P, H)),
        "vS4z": vz,
        "ident4": np.eye(GB, dtype=np.float16),
        "covB": np.ascontiguousarray(np.broadcast_to(
            (covc / P).astype(np.float16), (P, BLOC, Lc))),
        "biasPE": np.ascontiguousarray(
            biasf[s0:s0 + BLOC].T.reshape(PC, P, BLOC).transpose(1, 0, 2)),
        "mask4": m4,
    }


def kernel(encoder_outputs, attn_mask, hidden, coverage,
           attn_w, attn_b, dec_w, dec_b, cvg_w, cvg_b, v):
    enc = np.asarray(encoder_outputs, dtype=np.float32)
    mask = np.asarray(attn_mask)
    hidden = np.asarray(hidden, dtype=np.float32)
    coverage = np.asarray(coverage, dtype=np.float32)
    attn_w = np.asarray(attn_w, dtype=np.float32)
    attn_b = np.asarray(attn_b, dtype=np.float32)
    dec_w = np.asarray(dec_w, dtype=np.float32)
    dec_b = np.asarray(dec_b, dtype=np.float32)
    cvg_b = np.asarray(cvg_b, dtype=np.float32)
    v = np.asarray(v, dtype=np.float32)
    # 'same' padding with kernel (1, H) on a single pixel: only the center
    # column of the conv weight is ever active.
    center = (H - 1) // 2
    w_eff = np.asarray(cvg_w[:, :, 0, center], dtype=np.float32)
    # tiny linears precomputed host-side (0.2% of total FLOPs)
    covf = coverage @ w_eff.T + cvg_b                 # [B, L] cov_feat
    biasf = hidden @ dec_w.T + dec_b + attn_b         # [B, H] tanh bias

    # mask compaction: keep only mask==1 columns, pad to Lc
    idx_all = [np.nonzero(mask[b] == 1)[0] for b in range(B)]
    max_n = max(len(ix) for ix in idx_all)
    Lc = 384
    if max_n > Lc:
        Lc = L
        idx_all = [np.arange(L) for _ in range(B)]

    nc = _get_program(Lc)
    in_maps = [
        _prep_core_inputs(c, Lc, idx_all, enc, mask, coverage, attn_w, v,
                          covf, biasf)
        for c in range(NCORES)
    ]
    trace = os.environ.get("KERNEL_TRACE", "") == "1"
    res = run_bass_kernel_spmd(nc, in_maps, core_ids=list(range(NCORES)),
                               trace=trace)
    if trace and res.exec_time_ns is not None:
        _CACHE["exec_time_ns"] = res.exec_time_ns
        _CACHE["mean_exec_time_ns"] = res.mean_exec_time_ns
        _CACHE["trace"] = res.instructions_and_trace

    ctx = np.empty((B, H), np.float32)
    aw = np.zeros((B, L), np.float32)
    for c in range(NCORES):
        r = res.results[c]
        e_c = r["aw_out"].reshape(NG, GB, Lc).astype(np.float32)
        rec = 1.0 / r["se_out"].reshape(NG, GB, 1)
        aw_c = e_c * rec                      # normalized attn weights
        ctx[c * BLOC:(c + 1) * BLOC] = (
            r["ctx_out"].reshape(NG, GB, H) * rec).reshape(BLOC, H)
        for i in range(BLOC):
            gb = c * BLOC + i
            idx = idx_all[gb]
            n = len(idx)
            g, bi = divmod(i, GB)
            aw[gb, idx] = aw_c[g, bi, :n]
    ncov = coverage + aw
    return ctx, aw, ncov
